# revision 1
# baseline (speedup 1.0000x reference)
"""TRN2 Bass kernel for nn_DSSMEmbed (vq_codebook).

Strategy (8 NeuronCores, data-parallel over batch, 256 imgs/core):
  - Activation layout: partitions = (x, channel) rows, free = (y, img).
  - 3x3 convs as Toeplitz matmuls over x-windows with batch streamed in N;
    dy handled by PSUM accumulation at shifted free-dim (y) offsets.
  - emb conv: 64x32 8-tile mode, windowed one-hot input from DRAM (K=56).
  - c1 conv:  128x32 4-col-tile mode, K=128 direct from a duplicated
    3-chunk natural layout (x0..7 / x4..11 / x8..15) -- no window copies.
  - c2 conv:  64x32 8-tile mode, windowed y-pair buffers built by DMA.
  - Embedding lookup folded into emb conv: host ships windowed one-hot(s)
    (bf16, tower1) and one-hot(s')-one-hot(s) (fp32 delta, tower2); the
    renormed embedding table is folded into the conv operator on host.
  - Tower2 (feeds VQ argmax) entirely fp32; tower1 + final BxB matmul bf16.
  - VQ: scores.T via PE (fp32), per-row max/max_index on DVE, indirect-DMA
    gather of zn rows, PE transpose, AllGather, local bf16 (256,512)@(512,2048).
  - embed1 norms via ones-matmul of squares; 1/(|e|+eps) and exp(scale)
    folded into the final evacuation as per-partition scalars.
"""
import sys

sys.path.insert(0, "/opt/trn_rl_repo")

import numpy as np
import concourse.bass as bass
import concourse.bacc as bacc
import concourse.mybir as mybir
import concourse.tile as tile
from concourse.bass_utils import run_bass_kernel_spmd

F32 = mybir.dt.float32
BF16 = mybir.dt.bfloat16
U32 = mybir.dt.uint32
AF = mybir.ActivationFunctionType

NCORES = 8
B = 2048
BL = B // NCORES          # 256 imgs per core
H = W = 16
DICT, SE, CE, ESZ, NZ = 14, 8, 16, 512, 512
EPS = 1e-4
YB = H * BL               # free dim (y, img) = 4096

DEBUG = False

# ---------------------------------------------------------------------------
# host-side preprocessing
# ---------------------------------------------------------------------------


def make_windowed_oh(nat):
    """nat: (DICT, H, W, Bloc) one-hot -> (4, 4, 128, 6, Bloc).

    px=2: 8 blocks; tensor t holds block t at rows 0.. and block t+4 at
    rows 64..; rows w*14+d for window x' = 2b-1+w, w in 0..3.  Second dim
    is the y-quarter: quarter q covers global y in [4q-1, 4q+5) (clipped,
    duplicated halo) so each DMA load is contiguous per partition.
    """
    out = np.zeros((4, 4, 128, 6, nat.shape[-1]), dtype=np.int8)
    for b in range(8):
        t, h = b % 4, b // 4
        for w in range(4):
            xs = 2 * b - 1 + w
            if 0 <= xs < W:
                for q in range(4):
                    ys, ye = max(0, 4 * q - 1), min(H, 4 * q + 5)
                    out[t, q, h * 64 + w * DICT:h * 64 + (w + 1) * DICT,
                        ys - (4 * q - 1):ye - (4 * q - 1)] = nat[:, ys:ye, xs, :]
    return out


def op_emb_win(wfold):
    """Folded emb conv operator for 64x32 windowed scheme: (3, 4, 128, 32).

    wfold: (C_out=16, DICT, 3, 3).  lhsT[dy, t, h*64 + w*14 + d,
    xr*16 + co] = wfold[co, d, dy, w - xr] (dx = w - xr in 0..2).
    """
    op = np.zeros((3, 4, 128, 32), dtype=np.float32)
    for dy in range(3):
        blk = np.zeros((56, 32), dtype=np.float32)
        for w in range(4):
            for xr in range(2):
                dx = w - xr
                if 0 <= dx <= 2:
                    blk[w * DICT:(w + 1) * DICT, xr * 16:(xr + 1) * 16] = \
                        wfold[:, :, dy, dx].T
        for h in range(2):
            op[dy, :, h * 64:h * 64 + 56, :] = blk[None]
    return op


def op_conv_win(wc, c_in, c_out):
    """Windowed 64-row conv operator: (3, 4, 128, px*c_out) with px=2.

    wc: (c_out, c_in, 3, 3).  Tensor t serves blocks b=t (rows 0..) and
    b=t+4 (rows 64..); rows w*c_in+ci for window x' = 2b-1+w (w in 0..3),
    cols xr*c_out+co.  Boundary rows (x'=-1 for b=0, x'=16 for b=7) are
    zeroed (matching window zero padding).
    """
    M = 2 * c_out
    op = np.zeros((3, 4, 128, M), dtype=np.float32)
    blk = np.zeros((4 * c_in, M), dtype=np.float32)
    for dy in range(3):
        blk[:] = 0.0
        for w in range(4):
            for xr in range(2):
                dx = w - xr
                if 0 <= dx <= 2:
                    blk[w * c_in:(w + 1) * c_in, xr * c_out:(xr + 1) * c_out] = \
                        wc[:, :, dy, dx].T
        for h in range(2):
            op[dy, :, h * 64:h * 64 + 4 * c_in, :] = blk[None]
        op[dy, 0, 0:c_in, :] = 0.0                    # b=0, w=0 (x'=-1)
        op[dy, 3, 64 + 3 * c_in:64 + 4 * c_in, :] = 0.0  # b=7, w=3 (x'=16)
    return op


def host_prep(inputs):
    s = np.asarray(inputs["s"])
    sp = np.asarray(inputs["s_prime"])
    se_w = np.asarray(inputs["state_embed"], dtype=np.float32)
    norms = np.sqrt((se_w * se_w).sum(1, keepdims=True))
    table = se_w / np.maximum(norms, 1.0)

    oh_s = (np.arange(DICT)[:, None, None, None] ==
            s.transpose(1, 2, 0)[None]).astype(np.float32)
    oh_sp = (np.arange(DICT)[:, None, None, None] ==
             sp.transpose(1, 2, 0)[None]).astype(np.float32)
    oh_d = oh_sp - oh_s

    emb_fold = np.einsum("oikl,di->odkl",
                         np.asarray(inputs["conv_embed_w"], np.float32), table)

    shared = {
        "op_emb": op_emb_win(emb_fold),
        "op_c1t1": op_conv_win(np.asarray(inputs["p1c1_w"], np.float32), 16, 16),
        "op_c1t2": op_conv_win(np.asarray(inputs["p2c1_w"], np.float32), 16, 16),
        "op_c2t1": op_conv_win(np.asarray(inputs["p1c2_w"], np.float32), 16, 32),
        "op_c2t2": op_conv_win(np.asarray(inputs["p2c2_w"], np.float32), 16, 32),
    }

    def reorder_lin(lw):
        # K order: (chunk c, y, row r), r = xr*32+ch, x = c*4+xr
        lw = np.asarray(lw, np.float32).reshape(ESZ, 32, H, W)
        lw = lw.transpose(3, 1, 2, 0).reshape(4, 4, 32, H, ESZ)  # (c,xr,ch,y,E)
        return np.ascontiguousarray(
            lw.transpose(0, 3, 1, 2, 4).reshape(4, H, 128, ESZ).reshape(64, 128, ESZ))

    shared["lw_t1"] = reorder_lin(inputs["p1l_w"])
    shared["lw_t2"] = reorder_lin(inputs["p2l_w"])

    zv = np.asarray(inputs["z_vectors"], np.float32)
    zn = zv / np.sqrt((zv * zv).sum(1, keepdims=True))
    shared["zn"] = zn
    shared["znT"] = np.ascontiguousarray(zn.T)

    def conv_bias(bvec, c_out):
        reps = 128 // c_out
        return np.ascontiguousarray(
            np.tile(np.asarray(bvec, np.float32), reps)[:, None])

    shared["b_emb"] = conv_bias(inputs["conv_embed_b"], 16)
    shared["b_c1t1"] = conv_bias(inputs["p1c1_b"], 16)
    shared["b_c1t2"] = conv_bias(inputs["p2c1_b"], 16)
    shared["b_c2t1"] = conv_bias(inputs["p1c2_b"], 32)
    shared["b_c2t2"] = conv_bias(inputs["p2c2_b"], 32)
    shared["b_l1"] = np.ascontiguousarray(
        np.asarray(inputs["p1l_b"], np.float32).reshape(1, ESZ))
    shared["b_l2"] = np.ascontiguousarray(
        np.asarray(inputs["p2l_b"], np.float32).reshape(1, ESZ))

    esc = float(np.exp(np.asarray(inputs["scale"], np.float32).reshape(-1)[0]))

    percore = []
    for c in range(NCORES):
        sl = slice(c * BL, (c + 1) * BL)
        percore.append({
            "ohs": make_windowed_oh(oh_s[..., sl]),
            "ohd": make_windowed_oh(oh_d[..., sl]),
        })
    return shared, percore, esc


# ---------------------------------------------------------------------------
# device program
# ---------------------------------------------------------------------------


def _clip_dy(y0, ny, dy):
    s = max(y0, -dy)
    e = min(y0 + ny, H - dy)
    if s >= e:
        return None
    return (s - y0) * BL, (e - s) * BL, s + dy


def build_program(esc, debug=False):
    from contextlib import ExitStack
    nc = bacc.Bacc("TRN2", target_bir_lowering=False, debug=False,
                   num_devices=NCORES)

    def din(name, shape, dt):
        return nc.dram_tensor(name, list(shape), dt, kind="ExternalInput").ap()

    ohs_d = din("ohs", (4, 4, 128, 6, BL), mybir.dt.int8)
    ohd_d = din("ohd", (4, 4, 128, 6, BL), mybir.dt.int8)
    op_embt1_d = din("op_embt1", (3, 4, 128, 32), BF16)
    op_embt2_d = din("op_embt2", (3, 4, 128, 32), F32)
    op_c1t1_d = din("op_c1t1", (3, 4, 128, 32), BF16)
    op_c1t2_d = din("op_c1t2", (3, 4, 128, 32), F32)
    op_c2t1_d = din("op_c2t1", (3, 4, 128, 64), BF16)
    op_c2t2_d = din("op_c2t2", (3, 4, 128, 64), F32)
    lw1_d = din("lw1", (64, 128, ESZ), BF16)
    lw2_d = din("lw2", (64, 128, ESZ), F32)
    b_se_d = din("b_se", (128, 1), F32)
    b_c1t1_d = din("b_c1t1", (128, 1), F32)
    b_c1t2_d = din("b_c1t2", (128, 1), F32)
    b_c2t1_d = din("b_c2t1", (128, 1), F32)
    b_c2t2_d = din("b_c2t2", (128, 1), F32)
    b_l1_d = din("b_l1", (1, ESZ), F32)
    b_l2_d = din("b_l2", (1, ESZ), F32)
    znt_d = din("znt", (ESZ, NZ), F32)
    zn_d = din("zn", (NZ, ESZ), F32)
    ident_d = din("ident", (128, 128), F32)

    out_d = nc.dram_tensor("out", [BL, B], F32, kind="ExternalOutput").ap()
    dbg = {}
    if debug:
        for nm, shp, dt in [("dbg_e1", (2, 128, ESZ), F32),
                            ("dbg_e2", (4, 128, BL), F32),
                            ("dbg_sc", (2, 128, NZ), F32),
                            ("dbg_idx", (2, 128, 8), U32),
                            ("dbg_d", (2, 128, YB), F32),
                            ("dbg_se", (2, 128, YB), BF16),
                            ("dbg_c1a", (2, 128, YB), BF16),
                            ("dbg_c2a", (4, 128, YB), BF16),
                            ("dbg_c1", (2, 128, YB), F32),
                            ("dbg_c2", (4, 128, YB), F32)]:
            dbg[nm] = nc.dram_tensor(nm, list(shp), dt,
                                     kind="ExternalOutput").ap()

    zloc_d = nc.dram_tensor("zloc", [ESZ, BL], BF16).ap()
    zg_d = nc.dram_tensor("zg", [NCORES * ESZ, BL], BF16,
                          addr_space="Shared").ap()

    with tile.TileContext(nc) as tc, ExitStack() as ES:
        cst = ES.enter_context(tc.tile_pool(name="cst", bufs=1))
        npool = ES.enter_context(tc.tile_pool(name="nat", bufs=1))
        epool = ES.enter_context(tc.tile_pool(name="emb", bufs=1))

        ident_sb = cst.tile([128, 128], F32, tag="ident", name="ident")
        nc.sync.dma_start(ident_sb[:], ident_d[:])
        ones_sb = cst.tile([128, 1], F32, tag="ones", name="ones")
        nc.vector.memset(ones_sb[:], 1.0)
        bias_sb = {}
        for nm, d in [("b_se", b_se_d), ("b_c1t1", b_c1t1_d),
                      ("b_c1t2", b_c1t2_d), ("b_c2t1", b_c2t1_d),
                      ("b_c2t2", b_c2t2_d)]:
            t = cst.tile([128, 1], F32, tag=nm, name=nm)
            nc.sync.dma_start(t[:], d[:])
            bias_sb[nm] = t
        bl_sb = {}
        for nm, d in [("b_l1", b_l1_d), ("b_l2", b_l2_d)]:
            t = cst.tile([1, ESZ], F32, tag=f"{nm}r", name=f"{nm}r")
            nc.sync.dma_start(t[:], d[:])
            bl_sb[nm] = t
        ones_k = cst.tile([1, 128], F32, tag="ones_k", name="ones_k")
        nc.vector.memset(ones_k[:], 1.0)

        def load_ops(op_d, dt, width, nt, pfx):
            ops = [[cst.tile([128, width], dt, tag=f"{pfx}{dy}{t}",
                             name=f"{pfx}{dy}{t}") for t in range(nt)]
                   for dy in range(3)]
            for dy in range(3):
                for t in range(nt):
                    nc.sync.dma_start(ops[dy][t][:], op_d[dy, t])
            return ops

        ops_embt2 = load_ops(op_embt2_d, F32, 32, 4, "oe2")
        ops_embt1 = load_ops(op_embt1_d, BF16, 32, 4, "oe1")
        ops_c1t2 = load_ops(op_c1t2_d, F32, 32, 4, "oc12")
        ops_c1t1 = load_ops(op_c1t1_d, BF16, 32, 4, "oc11")
        ops_c2t2 = load_ops(op_c2t2_d, F32, 64, 4, "od12")
        ops_c2t1 = load_ops(op_c2t1_d, BF16, 64, 4, "od11")

        # ---------------- emb conv (64x32 8-tile, windowed DRAM input) ----
        def emb_conv(oh_d, ops, dt, bias, tags, wbufs=2):
            outs = [npool.tile([128, YB], dt, tag=tg, name=tg) for tg in tags]
            with tc.tile_pool(name=f"ew{tags[0]}", bufs=wbufs) as wp, \
                 tc.tile_pool(name=f"ep{tags[0]}", bufs=2, space="PSUM") as pp:
                for q in range(4):
                    wins = []
                    for t in range(4):
                        w = wp.tile([128, 6, BL], dt, tag=f"w{t}", name=f"w{t}")
                        nc.gpsimd.dma_start(w[:], oh_d[t, q])
                        wins.append(w)
                    for yg in (2 * q, 2 * q + 1):
                        y0 = 2 * yg
                        ps = [pp.tile([128, 2 * BL], F32, tag=f"p{i}", name=f"p{i}")
                              for i in range(2)]
                        first = True
                        for dy in (0, -1, 1):
                            n0, N, ysrc = _clip_dy(y0, 2, dy)
                            ly = ysrc - (4 * q - 1)
                            nys = N // BL
                            for b in range(8):
                                t, hh = b % 4, b // 4
                                nc.tensor.matmul(
                                    ps[hh][32 * (b % 4):32 * (b % 4) + 32,
                                           n0:n0 + N],
                                    ops[dy + 1][t][hh * 64:hh * 64 + 56, :],
                                    wins[t][hh * 64:hh * 64 + 56,
                                            ly:ly + nys, :],
                                    start=first, stop=(dy == 1),
                                    tile_position=(hh * 64, 32 * (b % 4)))
                            first = False
                        sl = slice(y0 * BL, (y0 + 2) * BL)
                        bb0 = bias[:] if bias is not None else 0.0
                        nc.scalar.activation(outs[0][:, sl], ps[0][:],
                                             AF.Identity, bias=bb0)
                        nc.scalar.activation(outs[1][:, sl], ps[1][:],
                                             AF.Identity, bias=bb0)
            return outs

        # -------- windowed x-pair builder: 2-chunk nat -> 4 win tensors ----
        def build_wins(nat2, dt, q, wp):
            """Window tensor t rows [h*64 + (x'-(2b-1))*16 + ci] with
            b = t + 4h, covering y-quarter q (global y in [4q-1, 4q+5))."""
            ys, ye = max(0, 4 * q - 1), min(H, 4 * q + 5)
            ly0, ly1 = ys - (4 * q - 1), ye - (4 * q - 1)
            wins = []
            for t in range(4):
                w = wp.tile([128, 6, BL], dt, tag=f"w{t}", name=f"w{t}")
                for hh in range(2):
                    b = t + 4 * hh
                    x0 = 2 * b - 1
                    if b == 0:
                        nc.vector.memset(w[0:64, :, :], 0.0)
                    if b == 7:
                        nc.vector.memset(w[64:128, :, :], 0.0)
                    xs_s, xs_e = max(0, x0), min(W, x0 + 4)
                    pieces = []
                    if xs_s < 8 < xs_e:
                        pieces = [(xs_s, 8), (8, xs_e)]
                    else:
                        pieces = [(xs_s, xs_e)]
                    for (a, bb) in pieces:
                        ch = a // 8
                        nc.sync.dma_start(
                            w[hh * 64 + (a - x0) * 16:hh * 64 + (bb - x0) * 16,
                              ly0:ly1, :],
                            nat2[ch].rearrange("p (y i) -> p y i", y=H)
                            [(a % 8) * 16:(a % 8) * 16 + (bb - a) * 16, ys:ye, :])
                wins.append(w)
            return wins

        # ---------------- c1 conv (64x32 8-tile, windowed) -----------------
        def c1_conv(ins2, ops, dt, bias, tags):
            outs = [npool.tile([128, YB], dt, tag=tg, name=tg) for tg in tags]
            with tc.tile_pool(name=f"cw{tags[0]}", bufs=2) as wp, \
                 tc.tile_pool(name=f"cp{tags[0]}", bufs=2, space="PSUM") as pp:
                for q in range(4):
                    wins = build_wins(ins2, dt, q, wp)
                    for yg in (2 * q, 2 * q + 1):
                        y0 = 2 * yg
                        ps = [pp.tile([128, 2 * BL], F32, tag=f"p{i}", name=f"p{i}")
                              for i in range(2)]
                        first = True
                        for dy in (0, -1, 1):
                            n0, N, ysrc = _clip_dy(y0, 2, dy)
                            ly = ysrc - (4 * q - 1)
                            nys = N // BL
                            for b in range(8):
                                t, hh = b % 4, b // 4
                                nc.tensor.matmul(
                                    ps[hh][32 * (b % 4):32 * (b % 4) + 32,
                                           n0:n0 + N],
                                    ops[dy + 1][t][hh * 64:hh * 64 + 64, :],
                                    wins[t][hh * 64:hh * 64 + 64, ly:ly + nys, :],
                                    start=first, stop=(dy == 1),
                                    tile_position=(hh * 64, 32 * (b % 4)))
                            first = False
                        sl = slice(y0 * BL, (y0 + 2) * BL)
                        for i in range(2):
                            nc.scalar.activation(outs[i][:, sl], ps[i][:],
                                                 AF.Relu, bias=bias[:])
            return outs

        # ---------------- c2 conv (64x64 4-tile, windowed) -----------------
        def c2_conv(ins2, ops, dt, bias, tags):
            outs = [npool.tile([128, YB], dt, tag=tg, name=tg) for tg in tags]
            BORD = [0, 1, 4, 5, 2, 3, 6, 7]
            with tc.tile_pool(name=f"dw{tags[0]}", bufs=2) as wp, \
                 tc.tile_pool(name=f"dp{tags[0]}", bufs=2, space="PSUM") as pp:
                for q in range(4):
                    wins = build_wins(ins2, dt, q, wp)
                    for yg in (2 * q, 2 * q + 1):
                        y0 = 2 * yg
                        ps = [pp.tile([128, 2 * BL], F32, tag=f"p{i}", name=f"p{i}")
                              for i in range(4)]
                        first = True
                        for dy in (0, -1, 1):
                            n0, N, ysrc = _clip_dy(y0, 2, dy)
                            ly = ysrc - (4 * q - 1)
                            nys = N // BL
                            for b in BORD:
                                t, hh = b % 4, b // 4
                                nc.tensor.matmul(
                                    ps[b // 2][64 * (b % 2):64 * (b % 2) + 64,
                                               n0:n0 + N],
                                    ops[dy + 1][t][hh * 64:hh * 64 + 64, :],
                                    wins[t][hh * 64:hh * 64 + 64, ly:ly + nys, :],
                                    start=first, stop=(dy == 1),
                                    tile_position=(hh * 64, 64 * (b % 2)))
                            first = False
                        sl = slice(y0 * BL, (y0 + 2) * BL)
                        for i in range(4):
                            nc.scalar.activation(outs[i][:, sl], ps[i][:],
                                                 AF.Relu, bias=bias[:])
            return outs

        # ---------------- linear (M=img, N=E; returns (img, E) chunks) ----
        def linear(c2o, lw_d, dt, bias_row, tagp, dma_eng=None):
            embT = [epool.tile([128, ESZ], F32, tag=f"{tagp}T{m}", name=f"{tagp}T{m}")
                    for m in range(2)]
            with tc.tile_pool(name=f"lw{tagp}", bufs=6) as lwp, \
                 tc.tile_pool(name=f"lp{tagp}", bufs=1, space="PSUM") as pp:
                ps = [pp.tile([128, ESZ], F32, tag=f"p{m}", name=f"p{m}")
                      for m in range(2)]
                for k in range(64):
                    cch, y = k // 16, k % 16
                    lwt = lwp.tile([128, ESZ], dt, tag="lw", name="lw")
                    (dma_eng or nc.scalar).dma_start(lwt[:], lw_d[k])
                    for m in range(2):
                        lhsT = c2o[cch][:, y * BL + 128 * m:y * BL + 128 * m + 128]
                        nc.tensor.matmul(ps[m][:], lhsT, lwt[:],
                                         start=(k == 0), stop=False)
                for m in range(2):
                    nc.tensor.matmul(ps[m][:], ones_k[:],
                                     bias_row[:], start=False, stop=True)
                for m in range(2):
                    nc.scalar.activation(embT[m][:], ps[m][:], AF.Identity)
            return embT

        def transpose_back(embT, dt, tagp):
            """(img,E) 2 chunks -> (E,img) 4 chunks of dtype dt."""
            emb = [epool.tile([128, BL], dt, tag=f"{tagp}{e}", name=f"{tagp}{e}")
                   for e in range(4)]
            with tc.tile_pool(name=f"tp{tagp}", bufs=2, space="PSUM") as tpp:
                for m in range(2):
                    for e in range(4):
                        tp = tpp.tile([128, 128], F32, tag="tp", name="tp")
                        nc.tensor.transpose(tp[:], embT[m][:, 128 * e:128 * e + 128],
                                            ident_sb[:])
                        nc.vector.tensor_copy(emb[e][:, 128 * m:128 * m + 128], tp[:])
            return emb

        # ================== tower 2 (fp32 delta path) ==================
        with nc.named_scope("t2emb"):
            d3 = emb_conv(ohd_d, ops_embt2, F32, None, ["A0", "A1"])
        if debug:
            for c in range(2):
                nc.sync.dma_start(dbg["dbg_d"][c], d3[c][:])
        with nc.named_scope("t2c1"):
            c1o2 = c1_conv(d3, ops_c1t2, F32, bias_sb["b_c1t2"],
                           ["B0", "B1"])
        if debug:
            for c in range(2):
                nc.sync.dma_start(dbg["dbg_c1"][c], c1o2[c][:])
        with nc.named_scope("t2c2"):
            c2o2 = c2_conv(c1o2, ops_c2t2, F32, bias_sb["b_c2t2"],
                           ["C0", "C1", "A0", "A1"])
        if debug:
            for c in range(4):
                nc.sync.dma_start(dbg["dbg_c2"][c], c2o2[c][:])
        with nc.named_scope("t2lin"):
            embT2 = linear(c2o2, lw2_d, F32, bl_sb["b_l2"], "e2")
            embed2 = transpose_back(embT2, F32, "e2n")
        if debug:
            for m in range(4):
                nc.sync.dma_start(dbg["dbg_e2"][m], embed2[m][:])

        # ================== VQ ==================
        # ================== tower 1 (bf16) ==================
        with nc.named_scope("t1emb"):
            se3 = emb_conv(ohs_d, ops_embt1, BF16, bias_sb["b_se"],
                           ["B0", "B1"])
        with nc.named_scope("t1c1"):
            c1o1 = c1_conv(se3, ops_c1t1, BF16, bias_sb["b_c1t1"],
                           ["C0", "C1"])
        ES.enter_context(nc.named_scope("vq"))
        with tc.tile_pool(name="vq", bufs=1) as vqp, \
             tc.tile_pool(name="vqp", bufs=1, space="PSUM") as vpp:
            pass_pool = None
            znt_sb = []
            for e in range(4):
                t = vqp.tile([128, NZ], F32, tag=f"znt{e}", name=f"znt{e}")
                nc.sync.dma_start(t[:], znt_d[128 * e:128 * e + 128, :])
                znt_sb.append(t)
            sps = [vpp.tile([128, NZ], F32, tag=f"s{m}", name=f"s{m}") for m in range(2)]
            for e in range(4):
                for m in range(2):
                    nc.tensor.matmul(sps[m][:],
                                     embed2[e][:, 128 * m:128 * m + 128],
                                     znt_sb[e][:], start=(e == 0),
                                     stop=(e == 3))
            idxs = []
            for m in range(2):
                sc = vqp.tile([128, NZ], F32, tag=f"sc{m}", name=f"sc{m}")
                nc.vector.tensor_copy(sc[:], sps[m][:])
                mx = vqp.tile([128, 8], F32, tag=f"mx{m}", name=f"mx{m}")
                nc.vector.max(mx[:], sc[:])
                ix = vqp.tile([128, 8], U32, tag=f"ix{m}", name=f"ix{m}")
                nc.vector.max_index(ix[:], mx[:], sc[:])
                idxs.append(ix)
                if debug:
                    nc.sync.dma_start(dbg["dbg_sc"][m], sc[:])
                    nc.sync.dma_start(dbg["dbg_idx"][m], ix[:])
            zt = [vqp.tile([128, BL], BF16, tag=f"zt{e}", name=f"zt{e}") for e in range(4)]
            for m in range(2):
                zl = vqp.tile([128, ESZ], F32, tag=f"zl{m}", name=f"zl{m}")
                nc.gpsimd.indirect_dma_start(
                    out=zl[:], out_offset=None, in_=zn_d[:],
                    in_offset=bass.IndirectOffsetOnAxis(ap=idxs[m][:, :1],
                                                        axis=0))
                for e in range(4):
                    tp = vpp.tile([128, 128], F32, tag="tp", name="tp")
                    nc.tensor.transpose(tp[:], zl[:, 128 * e:128 * e + 128],
                                        ident_sb[:])
                    nc.vector.tensor_copy(zt[e][:, 128 * m:128 * m + 128],
                                          tp[:])
            for e in range(4):
                nc.gpsimd.dma_start(zloc_d[128 * e:128 * e + 128, :], zt[e][:])
            nc.gpsimd.collective_compute(
                "AllGather", mybir.AluOpType.bypass,
                replica_groups=[list(range(NCORES))],
                ins=[zloc_d[:]], outs=[zg_d[:]])

        with nc.named_scope("t1c2"):
            c2o1 = c2_conv(c1o1, ops_c2t1, BF16, bias_sb["b_c2t1"],
                           ["A0", "A1", "B0", "B1"])
        with nc.named_scope("t1lin"):
            embT1 = linear(c2o1, lw1_d, BF16, bl_sb["b_l1"], "e1")
            e1b = transpose_back(embT1, BF16, "e1b")
        if debug:
            for c in range(2):
                nc.sync.dma_start(dbg["dbg_se"][c], se3[c][:])
            for c in range(2):
                nc.sync.dma_start(dbg["dbg_c1a"][c], c1o1[c][:])
            for c in range(4):
                nc.sync.dma_start(dbg["dbg_c2a"][c], c2o1[c][:])

        with tc.tile_pool(name="nrm", bufs=1) as nrp:
            rnt = [epool.tile([128, 1], F32, tag=f"rnt{m}", name=f"rnt{m}")
                   for m in range(2)]
            for m in range(2):
                sq = nrp.tile([128, ESZ], F32, tag="sq", name="sq")
                nc.vector.tensor_mul(sq[:], embT1[m][:], embT1[m][:])
                n2 = nrp.tile([128, 1], F32, tag="n2", name="n2")
                nc.vector.tensor_reduce(n2[:], sq[:], mybir.AxisListType.X,
                                        mybir.AluOpType.add)
                nc.scalar.sqrt(n2[:], n2[:])
                nc.vector.tensor_scalar_add(n2[:], n2[:], EPS)
                nc.vector.reciprocal(n2[:], n2[:])
                nc.vector.tensor_scalar_mul(rnt[m][:], n2[:], esc)

        # ================== final (bf16) ==================
        zgr = zg_d.rearrange("(c e p) i -> e p c i", c=NCORES, e=4, p=128)
        gsb = []
        for e in range(4):
            g = epool.tile([128, B], BF16, tag=f"g{e}", name=f"g{e}")
            nc.sync.dma_start(g[:], zgr[e])
            gsb.append(g)
        with tc.tile_pool(name="fin", bufs=1) as fqp, \
             tc.tile_pool(name="finp", bufs=2, space="PSUM") as fpp:
            osb = [fqp.tile([128, B], F32, tag=f"o{m}", name=f"o{m}") for m in range(2)]
            for n in range(4):
                for m in range(2):
                    fp = fpp.tile([128, 512], F32, tag=f"f{m}", name=f"f{m}")
                    for e in range(4):
                        nc.tensor.matmul(fp[:],
                                         e1b[e][:, 128 * m:128 * m + 128],
                                         gsb[e][:, 512 * n:512 * n + 512],
                                         start=(e == 0), stop=(e == 3))
                    nc.vector.tensor_scalar_mul(
                        osb[m][:, 512 * n:512 * n + 512], fp[:], rnt[m][:])
            for m in range(2):
                nc.sync.dma_start(out_d[128 * m:128 * m + 128, :], osb[m][:])

    nc.compile()
    return nc


def make_in_maps(shared, percore):
    import ml_dtypes
    bf = ml_dtypes.bfloat16

    def b16(x):
        return np.asarray(x, np.float32).astype(bf)

    base = {
        "op_embt1": b16(shared["op_emb"]),
        "op_embt2": np.ascontiguousarray(shared["op_emb"], np.float32),
        "op_c1t1": b16(shared["op_c1t1"]),
        "op_c1t2": np.ascontiguousarray(shared["op_c1t2"], np.float32),
        "op_c2t1": b16(shared["op_c2t1"]),
        "op_c2t2": np.ascontiguousarray(shared["op_c2t2"], np.float32),
        "lw1": b16(shared["lw_t1"]),
        "lw2": np.ascontiguousarray(shared["lw_t2"], np.float32),
        "b_se": shared["b_emb"], "b_c1t1": shared["b_c1t1"],
        "b_c1t2": shared["b_c1t2"], "b_c2t1": shared["b_c2t1"],
        "b_c2t2": shared["b_c2t2"],
        "b_l1": shared["b_l1"], "b_l2": shared["b_l2"],
        "znt": shared["znT"], "zn": shared["zn"],
        "ident": np.eye(128, dtype=np.float32),
    }
    maps = []
    for pc in percore:
        m = dict(base)
        m["ohs"] = np.ascontiguousarray(pc["ohs"])
        m["ohd"] = np.ascontiguousarray(pc["ohd"])
        maps.append(m)
    return maps


def kernel(**inputs):
    dsf = np.asarray(inputs.get("downscale_factor", 1)).reshape(-1)
    dsf = int(dsf[0]) if dsf.size else 1
    assert dsf == 1, f"only downscale_factor=1 supported, got {dsf}"
    shared, percore, esc = host_prep(inputs)
    nc = build_program(esc, debug=DEBUG)
    maps = make_in_maps(shared, percore)
    res = run_bass_kernel_spmd(nc, maps, list(range(NCORES)))
    out = np.concatenate([res.results[c]["out"] for c in range(NCORES)],
                         axis=0)
    return out.astype(np.float32)


KERNEL_RESULTS = {}


def run_for_test(inputs, trace=False):
    """test.py hook: returns (out, per-core results, BassKernelResults)."""
    shared, percore, esc = host_prep(inputs)
    nc = build_program(esc, debug=DEBUG)
    maps = make_in_maps(shared, percore)
    res = run_bass_kernel_spmd(nc, maps, list(range(NCORES)), trace=trace)
    out = np.concatenate([res.results[c]["out"] for c in range(NCORES)],
                         axis=0)
    return out.astype(np.float32), res



# revision 11
# speedup vs baseline: 1.2128x; 1.2128x over previous
"""TRN2 Bass kernel for nn_DSSMEmbed (vq_codebook), v2.

Strategy (8 NeuronCores, data-parallel over batch, 256 imgs/core):
  - Fold emb conv into c1 (both towers): 5x5 composite conv applied
    directly to windowed one-hot inputs (exact in bf16). A 15th "ones"
    channel carries the emb-bias border term.
  - Tower2 (feeds VQ argmax, needs ~fp32 scores): all matmuls are bf16
    with hi/lo splitting. Stage A: weights split hi+lo, one-hot moving
    exact -> 2 passes. c2 + fused scores: 3 passes (Wh*Xh, Wl*Xh, Wh*Xl).
  - Tower2 linear is fused with the VQ score matmul: M2 = lw.T @ znT
    precomputed on host; scores = X3 @ M2 + (b_l2 @ znT).
  - Tower1 all single-pass bf16. Final BxB product replaced by
    scores1T = zn @ embed1n.T (4 E-chunks x 4 nz-tiles x 256 imgs)
    + AllGather of uint32 code indices (8KB) + indirect-DMA row gather
    of scores1T rows -> out.T; host transposes per-core blocks.
  - Convs use a window-6/8 scheme: 4 x-outputs per matmul (M=64/128),
    halving instruction count vs the 2-x-output baseline.
"""
import sys

sys.path.insert(0, "/opt/trn_rl_repo")

import numpy as np
import concourse.bass as bass
import concourse.bacc as bacc
import concourse.mybir as mybir
import concourse.tile as tile
from concourse.bass_utils import run_bass_kernel_spmd

F32 = mybir.dt.float32
BF16 = mybir.dt.bfloat16
U32 = mybir.dt.uint32
AF = mybir.ActivationFunctionType

NCORES = 8
B = 2048
BL = B // NCORES          # 256 imgs per core
H = W = 16
DICT, SE, CE, ESZ, NZ = 14, 8, 16, 512, 512
NCH = DICT + 1            # +1 ones channel for emb-bias border term
EPS = 1e-4
YB = H * BL               # free dim (y, img) = 4096

# ---------------------------------------------------------------------------
# host-side preprocessing
# ---------------------------------------------------------------------------


def _hi(x):
    import ml_dtypes
    return np.asarray(x, np.float32).astype(ml_dtypes.bfloat16)


def _lo(x):
    import ml_dtypes
    x = np.asarray(x, np.float32)
    return (x - x.astype(ml_dtypes.bfloat16).astype(np.float32)).astype(
        ml_dtypes.bfloat16)


def make_win_onehot(nat):
    """nat: (NCH, H, W, Bloc) -> (4, 4, 128, 8, Bloc) int8 windows.

    Group g serves x-outs 4g..4g+3 via window x' = 4g-2+w, w in 0..7;
    rows w*NCH + c (120 used).  y-quarter q covers y in [4q-2, 4q+6)
    (halo 2); slot yy = y - (4q-2); out-of-image slots stay 0.
    """
    out = np.zeros((4, 4, 128, 8, nat.shape[-1]), dtype=np.int8)
    for g in range(4):
        for q in range(4):
            ys, ye = max(0, 4 * q - 2), min(H, 4 * q + 6)
            for w in range(8):
                xs = 4 * g - 2 + w
                if 0 <= xs < W:
                    out[g, q, w * NCH:(w + 1) * NCH,
                        ys - (4 * q - 2):ye - (4 * q - 2)] = nat[:, ys:ye, xs, :]
    return out


def stageA_ops(c1w, embfold, embb):
    """Cascade-exact composite operators for c1(emb(x)).

    Returns (main, corr_bot, corr_top):
      main:     (5, 4, 128, 64)  dy-tap operators (U = dy in 0..4)
      corr_bot: (4, 128, 64)     extra matmul for output row y=0
      corr_top: (4, 128, 64)     extra matmul for output row y=15

    Cascade semantics: out[y,x] = sum over intermediate offsets (a,b)
    with (y+a-1, x+b-1) inside the image of c1w[:,:,a,b] *
    emb_out[y+a-1, x+b-1]; emb_out = embfold (*) oh + embb.
    The b-restriction is x-position-specific -> folded into the
    g-specific op columns. The a-restriction only bites at y in {0,15}
    -> two correction matmuls reading the oh row y=0 / y=15 (only the
    u=2 / u=0 part of the phantom intermediate rows lands inside).
    The ones channel (c=NCH-1) carries embb through a centered 3x3
    validity-indicator kernel; it needs no restriction.
    """
    c1w = np.asarray(c1w, np.float32)
    embb = np.asarray(embb, np.float32)
    kb = np.einsum("ocuv,c->ouv", c1w, embb)  # (16, 3, 3)

    def bvalid(g, xr):
        x = 4 * g + xr
        lob = 1 if x == 0 else 0
        hib = 1 if x == W - 1 else 2
        return lob, hib

    main = np.zeros((5, 4, 128, 64), np.float32)
    for g in range(4):
        for xr in range(4):
            lob, hib = bvalid(g, xr)
            # composite taps: position (U=a+u, V=b+v), V = w - xr
            for a in range(3):
                for u in range(3):
                    U = a + u
                    for b in range(lob, hib + 1):
                        for v in range(3):
                            w = b + v + xr
                            if not (0 <= w < 8):
                                continue
                            main[U, g, w * NCH:w * NCH + DICT,
                                 xr * 16:(xr + 1) * 16] += np.einsum(
                                "oc,cd->do", c1w[:, :, a, b],
                                embfold[:, :, u, v])
            # ones channel: centered 3x3 kernel kb, no b-restriction
            for a in range(3):
                for b in range(3):
                    U = a + 1
                    w = b + 1 + xr
                    main[U, g, w * NCH + DICT, xr * 16:(xr + 1) * 16] += \
                        kb[:, a, b]

    corr_bot = np.zeros((4, 128, 64), np.float32)
    corr_top = np.zeros((4, 128, 64), np.float32)
    for g in range(4):
        for xr in range(4):
            lob, hib = bvalid(g, xr)
            for b in range(lob, hib + 1):
                for v in range(3):
                    w = b + v + xr
                    if not (0 <= w < 8):
                        continue
                    # y=0: drop (a=0) terms; phantom row enters via u=2
                    corr_bot[g, w * NCH:w * NCH + DICT,
                             xr * 16:(xr + 1) * 16] -= np.einsum(
                        "oc,cd->do", c1w[:, :, 0, b], embfold[:, :, 2, v])
                    # y=15: drop (a=2) terms; phantom row enters via u=0
                    corr_top[g, w * NCH:w * NCH + DICT,
                             xr * 16:(xr + 1) * 16] -= np.einsum(
                        "oc,cd->do", c1w[:, :, 2, b], embfold[:, :, 0, v])
    return main, corr_bot, corr_top


def op_c2(wc, dy):
    """3x3 c2 operator for one dy: (4, 128, 128) (g-independent).

    wc: (32, 16, 3, 3). lhsT[w*16 + ci, xr*32 + co] = wc[co, ci, dy, w-xr]
    for 0 <= w - xr <= 2 (x' = 4g-1+w, w in 0..5; x = 4g+xr).
    """
    blk = np.zeros((128, 128), np.float32)
    for w in range(6):
        for xr in range(4):
            dx = w - xr
            if 0 <= dx <= 2:
                blk[w * 16:(w + 1) * 16, xr * 32:(xr + 1) * 32] = \
                    wc[:, :, dy, dx].T
    return np.stack([blk] * 4)


def reorder_lin(lw):
    """(ESZ, 8192) -> (64, 128, ESZ): k-tile t=(g, y), row r = xr*32+ch,
    source index ch*256 + y*16 + (4g + xr)."""
    lw = np.asarray(lw, np.float32).reshape(-1, 32, H, W)  # (E, ch, y, x)
    E = lw.shape[0]
    lw = lw.transpose(3, 2, 1, 0).reshape(4, 4, H, 32, E)  # (g, xr, y, ch, E)
    lw = lw.transpose(0, 2, 1, 3, 4).reshape(4, H, 128, E)  # (g, y, xr*32+ch, E)
    return np.ascontiguousarray(lw.reshape(64, 128, E))


def host_prep(inputs):
    s = np.asarray(inputs["s"])
    sp = np.asarray(inputs["s_prime"])
    se_w = np.asarray(inputs["state_embed"], dtype=np.float32)
    norms = np.sqrt((se_w * se_w).sum(1, keepdims=True))
    table = se_w / np.maximum(norms, 1.0)
    embfold = np.einsum("oikl,di->odkl",
                        np.asarray(inputs["conv_embed_w"], np.float32), table)

    # one-hot (+ones ch) natural layout (NCH, H, W, Bloc) per core
    ar = np.arange(DICT)
    oh_s = (ar[:, None, None, None] == s.transpose(1, 2, 0)[None]).astype(
        np.int8)
    oh_sp = (ar[:, None, None, None] == sp.transpose(1, 2, 0)[None]).astype(
        np.int8)
    ones_row = np.ones((1, H, W, B), np.int8)
    nat_s = np.concatenate([oh_s, ones_row], axis=0)
    nat_d = np.concatenate([(oh_sp - oh_s), np.zeros_like(ones_row)], axis=0)

    opA_t1, cb_t1, ct_t1 = stageA_ops(inputs["p1c1_w"], embfold,
                                      inputs["conv_embed_b"])
    opA_t2, cb_t2, ct_t2 = stageA_ops(inputs["p2c1_w"], embfold,
                                      inputs["conv_embed_b"])
    opC_t1 = np.stack([op_c2(np.asarray(inputs["p1c2_w"], np.float32), dy)
                       for dy in range(3)])                      # (3,4,128,128)
    opC_t2 = np.stack([op_c2(np.asarray(inputs["p2c2_w"], np.float32), dy)
                       for dy in range(3)])
    # append corr ops as pseudo-dy slots 5 (bot) and 6 (top): (7,4,128,64)
    opA_t1 = np.concatenate([opA_t1, cb_t1[None], ct_t1[None]], axis=0)
    opA_t2 = np.concatenate([opA_t2, cb_t2[None], ct_t2[None]], axis=0)

    zv = np.asarray(inputs["z_vectors"], np.float32)
    zn = zv / np.sqrt((zv * zv).sum(1, keepdims=True))
    M2 = np.asarray(inputs["p2l_w"], np.float32).T @ zn.T  # (8192, NZ)
    M2re = reorder_lin(M2.T)                               # (64, 128, NZ)
    lw1re = reorder_lin(inputs["p1l_w"])                   # (64, 128, ESZ)
    brow = (np.asarray(inputs["p2l_b"], np.float32) @ zn.T).reshape(1, NZ)

    def conv_bias(bvec, c_out):
        reps = 128 // c_out
        return np.ascontiguousarray(
            np.tile(np.asarray(bvec, np.float32), reps)[:, None])

    shared = {
        "opA_t1": _hi(opA_t1),
        "opA_t2h": _hi(opA_t2), "opA_t2l": _lo(opA_t2),
        "opC_t1": _hi(opC_t1),
        "opC_t2h": _hi(opC_t2), "opC_t2l": _lo(opC_t2),
        "M2h": _hi(M2re), "M2l": _lo(M2re),
        "lw1": _hi(lw1re),
        "znTb": _hi(np.ascontiguousarray(
            zn.T.reshape(4, 128, NZ))),
        "b_c1t1": conv_bias(inputs["p1c1_b"], 16),
        "b_c1t2": conv_bias(inputs["p2c1_b"], 16),
        "b_c2t1": conv_bias(inputs["p1c2_b"], 32),
        "b_c2t2": conv_bias(inputs["p2c2_b"], 32),
        "b_l1": np.ascontiguousarray(
            np.asarray(inputs["p1l_b"], np.float32).reshape(1, ESZ)),
        "brow": np.ascontiguousarray(brow, np.float32),
        "ident": np.eye(128, dtype=np.float32),
    }
    esc = float(np.exp(np.asarray(inputs["scale"], np.float32).reshape(-1)[0]))

    percore = []
    for c in range(NCORES):
        sl = slice(c * BL, (c + 1) * BL)
        percore.append({
            "ohs": np.ascontiguousarray(make_win_onehot(nat_s[..., sl])),
            "ohd": np.ascontiguousarray(make_win_onehot(nat_d[..., sl])),
        })
    return shared, percore, esc


# ---------------------------------------------------------------------------
# device program
# ---------------------------------------------------------------------------


def _clip_dy(y0, ny, dy):
    s = max(y0, -dy)
    e = min(y0 + ny, H - dy)
    if s >= e:
        return None
    return (s - y0) * BL, (e - s) * BL, s + dy


def build_program(esc):
    from contextlib import ExitStack
    nc = bacc.Bacc("TRN2", target_bir_lowering=False, debug=False,
                   num_devices=NCORES)

    def din(name, shape, dt):
        return nc.dram_tensor(name, list(shape), dt, kind="ExternalInput").ap()

    ohs_d = din("ohs", (4, 4, 128, 8, BL), mybir.dt.int8)
    ohd_d = din("ohd", (4, 4, 128, 8, BL), mybir.dt.int8)
    opA_t1_d = din("opA_t1", (7, 4, 128, 64), BF16)
    opA_t2h_d = din("opA_t2h", (7, 4, 128, 64), BF16)
    opA_t2l_d = din("opA_t2l", (7, 4, 128, 64), BF16)
    opC_t1_d = din("opC_t1", (3, 4, 128, 128), BF16)
    opC_t2h_d = din("opC_t2h", (3, 4, 128, 128), BF16)
    opC_t2l_d = din("opC_t2l", (3, 4, 128, 128), BF16)
    M2h_d = din("M2h", (64, 128, NZ), BF16)
    M2l_d = din("M2l", (64, 128, NZ), BF16)
    lw1_d = din("lw1", (64, 128, ESZ), BF16)
    znTb_d = din("znTb", (4, 128, NZ), BF16)
    b_c1t1_d = din("b_c1t1", (128, 1), F32)
    b_c1t2_d = din("b_c1t2", (128, 1), F32)
    b_c2t1_d = din("b_c2t1", (128, 1), F32)
    b_c2t2_d = din("b_c2t2", (128, 1), F32)
    b_l1_d = din("b_l1", (1, ESZ), F32)
    brow_d = din("brow", (1, NZ), F32)
    ident_d = din("ident", (128, 128), F32)

    outT_d = nc.dram_tensor("out", [B, BL], F32, kind="ExternalOutput").ap()
    s1t_d = nc.dram_tensor("s1t", [NZ, BL], F32).ap()
    codes_loc_d = nc.dram_tensor("codes_loc", [BL], U32).ap()
    codes_g_d = nc.dram_tensor("codes_g", [B], U32, addr_space="Shared").ap()

    with tile.TileContext(nc) as tc, ExitStack() as ES:
        cst = ES.enter_context(tc.tile_pool(name="cst", bufs=1))
        npool = ES.enter_context(tc.tile_pool(name="nat", bufs=1))
        epool = ES.enter_context(tc.tile_pool(name="emb", bufs=1))

        ident_sb = cst.tile([128, 128], F32, tag="ident", name="ident")
        nc.sync.dma_start(ident_sb[:], ident_d[:])
        ones_k = cst.tile([1, 128], F32, tag="ones_k", name="ones_k")
        nc.vector.memset(ones_k[:], 1.0)
        bias_sb = {}
        for nm, d in [("b_c1t1", b_c1t1_d), ("b_c1t2", b_c1t2_d),
                      ("b_c2t1", b_c2t1_d), ("b_c2t2", b_c2t2_d)]:
            t = cst.tile([128, 1], F32, tag=nm, name=nm)
            nc.sync.dma_start(t[:], d[:])
            bias_sb[nm] = t
        bl_sb = {}
        for nm, d in [("b_l1", b_l1_d), ("brow", brow_d)]:
            t = cst.tile([1, max(ESZ, NZ)], F32, tag=f"{nm}r", name=f"{nm}r")
            nc.sync.dma_start(t[:], d[:])
            bl_sb[nm] = t

        def load_ops(op_d, ndy, width, pfx):
            ops = [[cst.tile([128, width], BF16, tag=f"{pfx}{dy}{g}",
                             name=f"{pfx}{dy}{g}") for g in range(4)]
                   for dy in range(ndy)]
            for dy in range(ndy):
                for g in range(4):
                    nc.sync.dma_start(ops[dy][g][:], op_d[dy, g])
            return ops

        opsA_t1 = load_ops(opA_t1_d, 7, 64, "a1")
        opsA_t2h = load_ops(opA_t2h_d, 7, 64, "a2h")
        opsA_t2l = load_ops(opA_t2l_d, 7, 64, "a2l")
        opsC_t1 = load_ops(opC_t1_d, 3, 128, "c1")
        opsC_t2h = load_ops(opC_t2h_d, 3, 128, "c2h")
        opsC_t2l = load_ops(opC_t2l_d, 3, 128, "c2l")
        znTb_sb = []
        for e in range(4):
            t = cst.tile([128, NZ], BF16, tag=f"znt{e}", name=f"znt{e}")
            nc.sync.dma_start(t[:], znTb_d[e])
            znTb_sb.append(t)

        # ---------------- stage A: composite 5x5 from one-hot windows ------
        def stageA(oh_d, op_list, bias, out_tags, hilo):
            """op_list: [opsH] or [opsH, opsL]. Returns hi [,lo] chunk pairs:
            each a list of 2 tiles (x0-7 / x8-15 rows (x%8)*16+co, free YB)."""
            outs = [[npool.tile([128, YB], BF16, tag=tg, name=tg)
                     for tg in tgs] for tgs in out_tags]
            with tc.tile_pool(name=f"Aw{out_tags[0][0]}", bufs=2) as wp, \
                 tc.tile_pool(name=f"At{out_tags[0][0]}", bufs=2) as tp, \
                 tc.tile_pool(name=f"Ap{out_tags[0][0]}", bufs=2,
                              space="PSUM") as pp:
                for q in range(4):
                    wins = []
                    for g in range(4):
                        w = wp.tile([128, 8, BL], BF16, tag=f"w{g}",
                                    name=f"w{g}")
                        (nc.gpsimd if g % 2 == 0 else nc.scalar).dma_start(
                            w[:], oh_d[g, q])
                        wins.append(w)
                    for yg in (2 * q, 2 * q + 1):
                        y0 = 2 * yg
                        ps = [pp.tile([128, 2 * BL], F32, tag=f"p{i}",
                                      name=f"p{i}") for i in range(2)]
                        mm = []
                        for dy in (0, -1, 1, -2, 2):
                            cl = _clip_dy(y0, 2, dy)
                            if cl is None:
                                continue
                            n0, N, ysrc = cl
                            ly = ysrc - (4 * q - 2)
                            nys = N // BL
                            for ops in op_list:
                                for g in range(4):
                                    mm.append((ops[dy + 2][g], g, n0, N, ly,
                                               nys))
                        if yg == 0:       # y=0 border correction (slot ly=2)
                            for ops in op_list:
                                for g in range(4):
                                    mm.append((ops[5][g], g, 0, BL, 2, 1))
                        if yg == 7:       # y=15 border correction (q=3 ly=5)
                            for ops in op_list:
                                for g in range(4):
                                    mm.append((ops[6][g], g, BL, BL, 5, 1))
                        first_g = {}
                        last_g = {}
                        for i, (op, g, n0, N, ly, nys) in enumerate(mm):
                            first_g.setdefault(g, i)
                            last_g[g] = i
                        for i, (op, g, n0, N, ly, nys) in enumerate(mm):
                            nc.tensor.matmul(
                                ps[g // 2][64 * (g % 2):64 * (g % 2) + 64,
                                           n0:n0 + N],
                                op[0:120, :],
                                wins[g][0:120, ly:ly + nys, :],
                                start=(i == first_g[g]),
                                stop=(i == last_g[g]),
                                tile_position=(0, 64 * (g % 2)))
                        sl = slice(y0 * BL, (y0 + 2) * BL)
                        for i in range(2):
                            if hilo:
                                tmp = tp.tile([128, 2 * BL], F32, tag=f"t{i}",
                                              name=f"t{i}")
                                nc.scalar.activation(tmp[:], ps[i][:],
                                                     AF.Relu, bias=bias[:])
                                nc.scalar.activation(outs[0][i][:, sl],
                                                     ps[i][:], AF.Relu,
                                                     bias=bias[:])
                                nc.vector.tensor_sub(outs[1][i][:, sl],
                                                     tmp[:], outs[0][i][:, sl])
                            else:
                                nc.scalar.activation(outs[0][i][:, sl],
                                                     ps[i][:], AF.Relu,
                                                     bias=bias[:])
            return outs

        # -------- c2 window builder: 2-chunk (x,16ch) -> 4 win tensors -----
        def build_wins_c2(src2, q, wp, pfx):
            """Window g rows w*16+ci, w in 0..5, x' = 4g-1+w; y in
            [4q-1, 4q+5) halo-1 slots."""
            ys, ye = max(0, 4 * q - 1), min(H, 4 * q + 5)
            ly0, ly1 = ys - (4 * q - 1), ye - (4 * q - 1)
            wins = []
            for g in range(4):
                w = wp.tile([128, 6, BL], BF16, tag=f"{pfx}w{g}",
                            name=f"{pfx}w{g}")
                x0 = 4 * g - 1
                if g == 0:
                    nc.vector.memset(w[0:32, :, :], 0.0)
                if g == 3:
                    nc.vector.memset(w[64:96, :, :], 0.0)
                xs_s, xs_e = max(0, x0), min(W, x0 + 6)
                if xs_s < 8 < xs_e:
                    pieces = [(xs_s, 8), (8, xs_e)]
                else:
                    pieces = [(xs_s, xs_e)]
                for (a, bb) in pieces:
                    ch = a // 8
                    nc.sync.dma_start(
                        w[(a - x0) * 16:(bb - x0) * 16, ly0:ly1, :],
                        src2[ch].rearrange("p (y i) -> p y i", y=H)
                        [(a % 8) * 16:(a % 8) * 16 + (bb - a) * 16, ys:ye, :])
                wins.append(w)
            return wins

        # ---------------- c2 conv (3x3, window-6, M=128) -------------------
        def c2_conv(srcs, op_list, bias, out_tags, hilo):
            """srcs: [Xh2] or [Xh2, Xl2] chunk pairs; passes:
            (opH, winH), (opL, winH), (opH, winL)."""
            outs = [[npool.tile([128, YB], BF16, tag=tg, name=tg)
                     for tg in tgs] for tgs in out_tags]
            with tc.tile_pool(name=f"Cw{out_tags[0][0]}", bufs=2) as wp, \
                 tc.tile_pool(name=f"Ct{out_tags[0][0]}", bufs=2) as tp, \
                 tc.tile_pool(name=f"Cp{out_tags[0][0]}", bufs=1,
                              space="PSUM") as pp:
                for q in range(4):
                    winsH = build_wins_c2(srcs[0], q, wp, "h")
                    winsL = build_wins_c2(srcs[1], q, wp, "l") if hilo else None
                    passes = [(op_list[0], winsH)]
                    if hilo:
                        passes += [(op_list[1], winsH), (op_list[0], winsL)]
                    for yg in (2 * q, 2 * q + 1):
                        y0 = 2 * yg
                        ps = [pp.tile([128, 2 * BL], F32, tag=f"p{g}",
                                      name=f"p{g}") for g in range(4)]
                        mm = []
                        for dy in (0, -1, 1):
                            cl = _clip_dy(y0, 2, dy)
                            if cl is None:
                                continue
                            n0, N, ysrc = cl
                            ly = ysrc - (4 * q - 1)
                            nys = N // BL
                            for (ops, wins) in passes:
                                for g in range(4):
                                    mm.append((ops[dy + 1][g], wins[g], g,
                                               n0, N, ly, nys))
                        first_g = {}
                        last_g = {}
                        for i, (op, win, g, n0, N, ly, nys) in enumerate(mm):
                            first_g.setdefault(g, i)
                            last_g[g] = i
                        for i, (op, win, g, n0, N, ly, nys) in enumerate(mm):
                            nc.tensor.matmul(
                                ps[g][:, n0:n0 + N],
                                op[0:96, :], win[0:96, ly:ly + nys, :],
                                start=(i == first_g[g]),
                                stop=(i == last_g[g]))
                        sl = slice(y0 * BL, (y0 + 2) * BL)
                        for g in range(4):
                            if hilo:
                                tmp = tp.tile([128, 2 * BL], F32, tag=f"t{g}",
                                              name=f"t{g}")
                                nc.scalar.activation(tmp[:], ps[g][:],
                                                     AF.Relu, bias=bias[:])
                                nc.scalar.activation(outs[0][g][:, sl],
                                                     ps[g][:], AF.Relu,
                                                     bias=bias[:])
                                nc.vector.tensor_sub(outs[1][g][:, sl],
                                                     tmp[:], outs[0][g][:, sl])
                            else:
                                nc.scalar.activation(outs[0][g][:, sl],
                                                     ps[g][:], AF.Relu,
                                                     bias=bias[:])
            return outs

        # ================== tower 2 ==================
        with nc.named_scope("t2A"):
            X2h, X2l = stageA(ohd_d, [opsA_t2h, opsA_t2l], bias_sb["b_c1t2"],
                              [["A0", "A1"], ["B0", "B1"]], hilo=True)
        with nc.named_scope("t2c2"):
            X3h, X3l = c2_conv([X2h, X2l], [opsC_t2h, opsC_t2l],
                               bias_sb["b_c2t2"],
                               [["C0", "C1", "C2", "C3"],
                                ["D0", "D1", "D2", "D3"]], hilo=True)

        # -------- fused scores: X3 @ M2 + brow; argmax -> codes ------------
        with nc.named_scope("t2sc"):
            with tc.tile_pool(name="m2p", bufs=8) as mwp, \
                 tc.tile_pool(name="scp", bufs=1) as scp, \
                 tc.tile_pool(name="spp", bufs=1, space="PSUM") as spp:
                sps = [spp.tile([128, NZ], F32, tag=f"s{m}", name=f"s{m}")
                       for m in range(2)]
                qs = [nc.scalar, nc.sync, nc.gpsimd]
                for k in range(64):
                    g, y = k // 16, k % 16
                    mh = mwp.tile([128, NZ], BF16, tag="mh", name="mh")
                    ml = mwp.tile([128, NZ], BF16, tag="ml", name="ml")
                    qs[k % 3].dma_start(mh[:], M2h_d[k])
                    qs[(k + 1) % 3].dma_start(ml[:], M2l_d[k])
                    for m in range(2):
                        c0 = y * BL + 128 * m
                        nc.tensor.matmul(sps[m][:],
                                         X3h[g][:, c0:c0 + 128], mh[:],
                                         start=(k == 0), stop=False)
                        nc.tensor.matmul(sps[m][:],
                                         X3l[g][:, c0:c0 + 128], mh[:],
                                         start=False, stop=False)
                        nc.tensor.matmul(sps[m][:],
                                         X3h[g][:, c0:c0 + 128], ml[:],
                                         start=False, stop=False)
                for m in range(2):
                    nc.tensor.matmul(sps[m][:], ones_k[:],
                                     bl_sb["brow"][:, 0:NZ], start=False,
                                     stop=True)
                idxs = []
                for m in range(2):
                    sc = scp.tile([128, NZ], F32, tag=f"sc{m}", name=f"sc{m}")
                    nc.vector.tensor_copy(sc[:], sps[m][:])
                    mx = scp.tile([128, 8], F32, tag=f"mx{m}", name=f"mx{m}")
                    nc.vector.max(mx[:], sc[:])
                    ix = scp.tile([128, 8], U32, tag=f"ix{m}", name=f"ix{m}")
                    nc.vector.max_index(ix[:], mx[:], sc[:])
                    idxs.append(ix)
                for m in range(2):
                    nc.gpsimd.dma_start(codes_loc_d[128 * m:128 * m + 128],
                                        idxs[m][:, 0:1])
            nc.gpsimd.collective_compute(
                "AllGather", mybir.AluOpType.bypass,
                replica_groups=[list(range(NCORES))],
                ins=[codes_loc_d[:]], outs=[codes_g_d[:]])

        # ================== tower 1 (bf16) ==================
        with nc.named_scope("t1A"):
            (Y2,) = stageA(ohs_d, [opsA_t1], bias_sb["b_c1t1"],
                           [["A0", "A1"]], hilo=False)
        with nc.named_scope("t1c2"):
            (Y3,) = c2_conv([Y2, None], [opsC_t1], bias_sb["b_c2t1"],
                            [["B0", "B1", "C0", "C1"]], hilo=False)

        # ---------------- t1 linear -> embT1 (img, ESZ) --------------------
        with nc.named_scope("t1lin"):
            embT1 = [epool.tile([128, ESZ], F32, tag=f"e1T{m}",
                                name=f"e1T{m}") for m in range(2)]
            with tc.tile_pool(name="lwp", bufs=6) as lwp, \
                 tc.tile_pool(name="lpp", bufs=1, space="PSUM") as lpp:
                ps = [lpp.tile([128, ESZ], F32, tag=f"p{m}", name=f"p{m}")
                      for m in range(2)]
                for k in range(64):
                    g, y = k // 16, k % 16
                    lwt = lwp.tile([128, ESZ], BF16, tag="lw", name="lw")
                    (nc.scalar if k % 2 == 0 else nc.sync).dma_start(
                        lwt[:], lw1_d[k])
                    for m in range(2):
                        c0 = y * BL + 128 * m
                        nc.tensor.matmul(ps[m][:], Y3[g][:, c0:c0 + 128],
                                         lwt[:], start=(k == 0), stop=False)
                for m in range(2):
                    nc.tensor.matmul(ps[m][:], ones_k[:], bl_sb["b_l1"][:, 0:ESZ],
                                     start=False, stop=True)
                for m in range(2):
                    nc.scalar.activation(embT1[m][:], ps[m][:], AF.Identity)

            # rnt = exp(scale) / (|e1| + eps); scale embT1 rows
            with tc.tile_pool(name="nrm", bufs=1) as nrp:
                for m in range(2):
                    sq = nrp.tile([128, ESZ], F32, tag="sq", name="sq")
                    nc.vector.tensor_mul(sq[:], embT1[m][:], embT1[m][:])
                    n2 = nrp.tile([128, 1], F32, tag="n2", name="n2")
                    nc.vector.tensor_reduce(n2[:], sq[:],
                                            mybir.AxisListType.X,
                                            mybir.AluOpType.add)
                    nc.scalar.sqrt(n2[:], n2[:])
                    nc.vector.tensor_scalar_add(n2[:], n2[:], EPS)
                    nc.vector.reciprocal(n2[:], n2[:])
                    nc.vector.tensor_scalar_mul(n2[:], n2[:], esc)
                    nc.vector.tensor_scalar_mul(embT1[m][:], embT1[m][:],
                                                n2[:])

            # transpose to e1b (E, img) bf16
            e1b = [epool.tile([128, BL], BF16, tag=f"e1b{e}", name=f"e1b{e}")
                   for e in range(4)]
            with tc.tile_pool(name="tpp", bufs=2, space="PSUM") as tpp:
                for m in range(2):
                    for e in range(4):
                        tp = tpp.tile([128, 128], F32, tag="tp", name="tp")
                        nc.tensor.transpose(tp[:],
                                            embT1[m][:, 128 * e:128 * e + 128],
                                            ident_sb[:])
                        nc.vector.tensor_copy(e1b[e][:, 128 * m:128 * m + 128],
                                              tp[:])

        # ---------------- scores1T = zn @ e1n.T -> s1t_d -------------------
        with nc.named_scope("fin"):
            with tc.tile_pool(name="s1p", bufs=1) as s1p, \
                 tc.tile_pool(name="fpp", bufs=2, space="PSUM") as fpp:
                for nt in range(4):
                    fp = fpp.tile([128, BL], F32, tag="fp", name="fp")
                    for e in range(4):
                        nc.tensor.matmul(
                            fp[:], znTb_sb[e][:, 128 * nt:128 * nt + 128],
                            e1b[e][:], start=(e == 0), stop=(e == 3))
                    s1 = s1p.tile([128, BL], F32, tag=f"s1{nt}",
                                  name=f"s1{nt}")
                    nc.scalar.activation(s1[:], fp[:], AF.Identity)
                    nc.sync.dma_start(s1t_d[128 * nt:128 * nt + 128, :],
                                      s1[:])

            # gather rows of s1t by global codes -> outT
            with tc.tile_pool(name="gp", bufs=4) as gp:
                codes_sb = gp.tile([128, 16], U32, tag="codes", name="codes")
                nc.sync.dma_start(codes_sb[:],
                                  codes_g_d.rearrange("(c p) -> p c", p=128))
                for c in range(16):
                    gt = gp.tile([128, BL], F32, tag="gt", name="gt")
                    nc.gpsimd.indirect_dma_start(
                        out=gt[:], out_offset=None, in_=s1t_d[:],
                        in_offset=bass.IndirectOffsetOnAxis(
                            ap=codes_sb[:, c:c + 1], axis=0))
                    nc.sync.dma_start(outT_d[128 * c:128 * c + 128, :],
                                      gt[:])

    nc.compile()
    return nc


def make_in_maps(shared, percore):
    base = dict(shared)
    maps = []
    for pc in percore:
        m = dict(base)
        m["ohs"] = pc["ohs"]
        m["ohd"] = pc["ohd"]
        maps.append(m)
    return maps


def _run(inputs, trace=False):
    dsf = np.asarray(inputs.get("downscale_factor", 1)).reshape(-1)
    dsf = int(dsf[0]) if dsf.size else 1
    assert dsf == 1, f"only downscale_factor=1 supported, got {dsf}"
    shared, percore, esc = host_prep(inputs)
    nc = build_program(esc)
    maps = make_in_maps(shared, percore)
    res = run_bass_kernel_spmd(nc, maps, list(range(NCORES)), trace=trace)
    out = np.concatenate(
        [res.results[c]["out"].T for c in range(NCORES)], axis=0)
    return np.ascontiguousarray(out, np.float32), res


def kernel(**inputs):
    out, _ = _run(inputs, trace=False)
    return out


def run_for_test(inputs, trace=False):
    return _run(inputs, trace=trace)


# revision 18
# speedup vs baseline: 1.2246x; 1.0097x over previous
"""TRN2 Bass kernel for nn_DSSMEmbed (vq_codebook), v3.

Strategy (8 NeuronCores, data-parallel over batch, 256 imgs/core):
  - Fold emb conv into c1 (both towers): cascade-exact composite 5x5
    operators applied to windowed one-hot inputs (exact in bf16), with
    x-border restrictions folded into per-group ops, y-border handled by
    two correction matmuls, and a 15th "ones" channel carrying the
    emb-bias validity term.
  - Tower2 (feeds VQ argmax, needs ~fp32 scores): all matmuls bf16 with
    hi/lo weight/activation splitting (2-pass stage A with exact one-hot
    moving operand; 3-pass c2 and fused scores).
  - Tower2 linear fused with VQ scoring: M2 = lw2.T @ znT on host;
    scores = X3 @ M2 + b_l2 @ znT. Streamed M2h on sync queue, M2l on
    scalar queue (the two hardware DGEs).
  - Tower1 single-pass bf16; final BxB product via AllGather of uint32
    codes (8KB) + indirect-DMA gather of zn rows + PE transpose +
    local (256, 2048) matmul.
  - Convs: window-6/8 scheme (4 x-outputs per matmul), full-y window
    tiles (no y-halo duplication), c2 loops g-outer so window builds
    pipeline with compute.
"""
import sys

sys.path.insert(0, "/opt/trn_rl_repo")

import numpy as np
import concourse.bass as bass
import concourse.bacc as bacc
import concourse.mybir as mybir
import concourse.tile as tile
from concourse.bass_utils import run_bass_kernel_spmd

F32 = mybir.dt.float32
BF16 = mybir.dt.bfloat16
U32 = mybir.dt.uint32
AF = mybir.ActivationFunctionType

NCORES = 8
B = 2048
BL = B // NCORES          # 256 imgs per core
H = W = 16
DICT, SE, CE, ESZ, NZ = 14, 8, 16, 512, 512
NCH = DICT + 1            # +1 ones channel for emb-bias border term
EPS = 1e-4
YB = H * BL               # free dim (y, img) = 4096

# ---------------------------------------------------------------------------
# host-side preprocessing
# ---------------------------------------------------------------------------


def _hi(x):
    import ml_dtypes
    return np.asarray(x, np.float32).astype(ml_dtypes.bfloat16)


def _lo(x):
    import ml_dtypes
    x = np.asarray(x, np.float32)
    return (x - x.astype(ml_dtypes.bfloat16).astype(np.float32)).astype(
        ml_dtypes.bfloat16)


def make_win_onehot(nat):
    """nat: (NCH, H, W, Bloc) -> (4, 128, H, Bloc) int8 full-y windows.

    Group g serves x-outs 4g..4g+3 via window x' = 4g-2+w, w in 0..7;
    rows w*NCH + c (120 used). Out-of-image x' rows stay 0.
    """
    import ml_dtypes
    out = np.zeros((4, 128, H, nat.shape[-1]), dtype=ml_dtypes.bfloat16)
    for g in range(4):
        for w in range(8):
            xs = 4 * g - 2 + w
            if 0 <= xs < W:
                out[g, w * NCH:(w + 1) * NCH] = nat[:, :, xs, :]
    return out


def stageA_ops(c1w, embfold, embb):
    """Cascade-exact composite operators for c1(emb(x)).

    Returns (5+2, 4, 128, 64): dy-tap operators (U in 0..4) followed by
    corr_bot (output row y=0) and corr_top (y=15) correction operators.

    Cascade semantics: out[y,x] = sum over intermediate offsets (a,b)
    with (y+a-1, x+b-1) inside the image of c1w[:,:,a,b] *
    emb_out[y+a-1, x+b-1]; emb_out = embfold (*) oh + embb.
    The b-restriction is x-position-specific -> folded into the
    g-specific op columns. The a-restriction only bites at y in {0,15}
    -> two correction matmuls reading the oh row y=0 / y=15 (only the
    u=2 / u=0 part of the phantom intermediate rows lands inside).
    The ones channel (c=NCH-1) carries embb through a centered 3x3
    validity-indicator kernel; it needs no restriction.
    """
    c1w = np.asarray(c1w, np.float32)
    embb = np.asarray(embb, np.float32)
    kb = np.einsum("ocuv,c->ouv", c1w, embb)  # (16, 3, 3)

    def bvalid(g, xr):
        x = 4 * g + xr
        lob = 1 if x == 0 else 0
        hib = 1 if x == W - 1 else 2
        return lob, hib

    ops = np.zeros((7, 4, 128, 64), np.float32)
    for g in range(4):
        for xr in range(4):
            lob, hib = bvalid(g, xr)
            for a in range(3):
                for u in range(3):
                    U = a + u
                    for b in range(lob, hib + 1):
                        for v in range(3):
                            w = b + v + xr
                            if not (0 <= w < 8):
                                continue
                            ops[U, g, w * NCH:w * NCH + DICT,
                                xr * 16:(xr + 1) * 16] += np.einsum(
                                "oc,cd->do", c1w[:, :, a, b],
                                embfold[:, :, u, v])
            # ones channel: centered 3x3 kernel kb, no b-restriction
            for a in range(3):
                for b in range(3):
                    ops[a + 1, g, (b + 1 + xr) * NCH + DICT,
                        xr * 16:(xr + 1) * 16] += kb[:, a, b]
            # y-border corrections
            for b in range(lob, hib + 1):
                for v in range(3):
                    w = b + v + xr
                    if not (0 <= w < 8):
                        continue
                    ops[5, g, w * NCH:w * NCH + DICT,
                        xr * 16:(xr + 1) * 16] -= np.einsum(
                        "oc,cd->do", c1w[:, :, 0, b], embfold[:, :, 2, v])
                    ops[6, g, w * NCH:w * NCH + DICT,
                        xr * 16:(xr + 1) * 16] -= np.einsum(
                        "oc,cd->do", c1w[:, :, 2, b], embfold[:, :, 0, v])
    return ops


def op_c2(wc, dy):
    """3x3 c2 operator for one dy: (4, 128, 128) (g-independent).

    wc: (32, 16, 3, 3). lhsT[w*16 + ci, xr*32 + co] = wc[co, ci, dy, w-xr]
    for 0 <= w - xr <= 2 (x' = 4g-1+w, w in 0..5; x = 4g+xr).
    """
    blk = np.zeros((128, 128), np.float32)
    for w in range(6):
        for xr in range(4):
            dx = w - xr
            if 0 <= dx <= 2:
                blk[w * 16:(w + 1) * 16, xr * 32:(xr + 1) * 32] = \
                    wc[:, :, dy, dx].T
    return np.stack([blk] * 4)


def reorder_lin(lw):
    """(ESZ, 8192) -> (64, 128, ESZ): k-tile t=(g, y), row r = xr*32+ch,
    source index ch*256 + y*16 + (4g + xr)."""
    lw = np.asarray(lw, np.float32).reshape(-1, 32, H, W)  # (E, ch, y, x)
    E = lw.shape[0]
    lw = lw.transpose(3, 2, 1, 0).reshape(4, 4, H, 32, E)  # (g, xr, y, ch, E)
    lw = lw.transpose(0, 2, 1, 3, 4).reshape(4, H, 128, E)
    return np.ascontiguousarray(lw.reshape(64, 128, E))


def host_prep(inputs):
    s = np.asarray(inputs["s"])
    sp = np.asarray(inputs["s_prime"])
    se_w = np.asarray(inputs["state_embed"], dtype=np.float32)
    norms = np.sqrt((se_w * se_w).sum(1, keepdims=True))
    table = se_w / np.maximum(norms, 1.0)
    embfold = np.einsum("oikl,di->odkl",
                        np.asarray(inputs["conv_embed_w"], np.float32), table)

    ar = np.arange(DICT)
    oh_s = (ar[:, None, None, None] == s.transpose(1, 2, 0)[None]).astype(
        np.int8)
    oh_sp = (ar[:, None, None, None] == sp.transpose(1, 2, 0)[None]).astype(
        np.int8)
    ones_row = np.ones((1, H, W, B), np.int8)
    nat_s = np.concatenate([oh_s, ones_row], axis=0)
    nat_d = np.concatenate([(oh_sp - oh_s), np.zeros_like(ones_row)], axis=0)

    opA_t1 = stageA_ops(inputs["p1c1_w"], embfold, inputs["conv_embed_b"])
    opA_t2 = stageA_ops(inputs["p2c1_w"], embfold, inputs["conv_embed_b"])
    opC_t1 = np.stack([op_c2(np.asarray(inputs["p1c2_w"], np.float32), dy)
                       for dy in range(3)])                      # (3,4,128,128)
    opC_t2 = np.stack([op_c2(np.asarray(inputs["p2c2_w"], np.float32), dy)
                       for dy in range(3)])

    zv = np.asarray(inputs["z_vectors"], np.float32)
    zn = zv / np.sqrt((zv * zv).sum(1, keepdims=True))
    M2 = np.asarray(inputs["p2l_w"], np.float32).T @ zn.T  # (8192, NZ)
    M2re = reorder_lin(M2.T)                               # (64, 128, NZ)
    lw1re = reorder_lin(inputs["p1l_w"])                   # (64, 128, ESZ)
    brow = (np.asarray(inputs["p2l_b"], np.float32) @ zn.T).reshape(1, NZ)

    def conv_bias(bvec, c_out):
        reps = 128 // c_out
        return np.ascontiguousarray(
            np.tile(np.asarray(bvec, np.float32), reps)[:, None])

    shared = {
        "opA_t1": _hi(opA_t1),
        "opA_t2h": _hi(opA_t2), "opA_t2l": _lo(opA_t2),
        "opC_t1": _hi(opC_t1),
        "opC_t2h": _hi(opC_t2), "opC_t2l": _lo(opC_t2),
        "M2h": _hi(M2re), "M2l": _lo(M2re),
        "lw1": _hi(lw1re),
        "znb": _hi(zn),
        "b_c1t1": conv_bias(inputs["p1c1_b"], 16),
        "b_c1t2": conv_bias(inputs["p2c1_b"], 16),
        "b_c2t1": conv_bias(inputs["p1c2_b"], 32),
        "b_c2t2": conv_bias(inputs["p2c2_b"], 32),
        "b_l1": np.ascontiguousarray(
            np.asarray(inputs["p1l_b"], np.float32).reshape(1, ESZ)),
        "brow": np.ascontiguousarray(brow, np.float32),
        "ident": np.eye(128, dtype=np.float32),
    }
    esc = float(np.exp(np.asarray(inputs["scale"], np.float32).reshape(-1)[0]))

    percore = []
    for c in range(NCORES):
        sl = slice(c * BL, (c + 1) * BL)
        percore.append({
            "ohs": np.ascontiguousarray(make_win_onehot(nat_s[..., sl])),
            "ohd": np.ascontiguousarray(make_win_onehot(nat_d[..., sl])),
        })
    return shared, percore, esc


# ---------------------------------------------------------------------------
# device program
# ---------------------------------------------------------------------------


def _clip_dy(y0, ny, dy):
    s = max(y0, -dy)
    e = min(y0 + ny, H - dy)
    if s >= e:
        return None
    return (s - y0) * BL, (e - s) * BL, s + dy


def build_program(esc):
    from contextlib import ExitStack
    nc = bacc.Bacc("TRN2", target_bir_lowering=False, debug=False,
                   num_devices=NCORES)

    def din(name, shape, dt):
        return nc.dram_tensor(name, list(shape), dt, kind="ExternalInput").ap()

    ohs_d = din("ohs", (4, 128, H, BL), BF16)
    ohd_d = din("ohd", (4, 128, H, BL), BF16)
    opA_t1_d = din("opA_t1", (7, 4, 128, 64), BF16)
    opA_t2h_d = din("opA_t2h", (7, 4, 128, 64), BF16)
    opA_t2l_d = din("opA_t2l", (7, 4, 128, 64), BF16)
    opC_t1_d = din("opC_t1", (3, 4, 128, 128), BF16)
    opC_t2h_d = din("opC_t2h", (3, 4, 128, 128), BF16)
    opC_t2l_d = din("opC_t2l", (3, 4, 128, 128), BF16)
    M2h_d = din("M2h", (64, 128, NZ), BF16)
    M2l_d = din("M2l", (64, 128, NZ), BF16)
    lw1_d = din("lw1", (64, 128, ESZ), BF16)
    znb_d = din("znb", (NZ, ESZ), BF16)
    b_c1t1_d = din("b_c1t1", (128, 1), F32)
    b_c1t2_d = din("b_c1t2", (128, 1), F32)
    b_c2t1_d = din("b_c2t1", (128, 1), F32)
    b_c2t2_d = din("b_c2t2", (128, 1), F32)
    b_l1_d = din("b_l1", (1, ESZ), F32)
    brow_d = din("brow", (1, NZ), F32)
    ident_d = din("ident", (128, 128), F32)

    out_d = nc.dram_tensor("out", [BL, B], F32, kind="ExternalOutput").ap()
    codes_loc_d = nc.dram_tensor("codes_loc", [BL], U32).ap()
    codes_g_d = nc.dram_tensor("codes_g", [B], U32, addr_space="Shared").ap()

    with tile.TileContext(nc) as tc, ExitStack() as ES:
        cst = ES.enter_context(tc.tile_pool(name="cst", bufs=1))
        npool = ES.enter_context(tc.tile_pool(name="nat", bufs=1))
        epool = ES.enter_context(tc.tile_pool(name="emb", bufs=1))

        # ---- issue t2 stage-A window loads FIRST (cold-start critical) ----
        # windows time-share npool YB-tile slots (a [128,H,BL] bf16 window
        # is the same size as a [128,YB] bf16 activation tile)
        def load_winA(oh_d, tags, engs):
            wins = []
            for g in range(4):
                w = npool.tile([128, H, BL], BF16, tag=tags[g],
                               name=f"win{tags[g]}")
                engs[g % len(engs)].dma_start(w[:], oh_d[g])
                wins.append(w)
            return wins

        wins_t2 = load_winA(ohd_d, ["C0", "C1", "C2", "C3"],
                            [nc.gpsimd, nc.scalar, nc.sync])

        ident_sb = cst.tile([128, 128], F32, tag="ident", name="ident")
        nc.sync.dma_start(ident_sb[:], ident_d[:])
        ones_k = cst.tile([1, 128], F32, tag="ones_k", name="ones_k")
        nc.vector.memset(ones_k[:], 1.0)
        bias_sb = {}
        for nm, d in [("b_c1t1", b_c1t1_d), ("b_c1t2", b_c1t2_d),
                      ("b_c2t1", b_c2t1_d), ("b_c2t2", b_c2t2_d)]:
            t = cst.tile([128, 1], F32, tag=nm, name=nm)
            nc.sync.dma_start(t[:], d[:])
            bias_sb[nm] = t
        bl_sb = {}
        for nm, d in [("b_l1", b_l1_d), ("brow", brow_d)]:
            t = cst.tile([1, ESZ], F32, tag=f"{nm}r", name=f"{nm}r")
            nc.sync.dma_start(t[:], d[:])
            bl_sb[nm] = t

        def load_ops(op_d, ndy, width, pfx, eng):
            ops = [[cst.tile([128, width], BF16, tag=f"{pfx}{dy}{g}",
                             name=f"{pfx}{dy}{g}") for g in range(4)]
                   for dy in range(ndy)]
            for dy in range(ndy):
                for g in range(4):
                    eng.dma_start(ops[dy][g][:], op_d[dy, g])
            return ops

        opsA_t2h = load_ops(opA_t2h_d, 7, 64, "a2h", nc.scalar)
        opsA_t2l = load_ops(opA_t2l_d, 7, 64, "a2l", nc.scalar)
        opsA_t1 = load_ops(opA_t1_d, 7, 64, "a1", nc.sync)
        opsC_t2h = load_ops(opC_t2h_d, 3, 128, "c2h", nc.scalar)
        opsC_t2l = load_ops(opC_t2l_d, 3, 128, "c2l", nc.scalar)
        opsC_t1 = load_ops(opC_t1_d, 3, 128, "c1", nc.sync)

        # ---------------- stage A: composite 5x5 from one-hot windows ------
        def stageA(wins, op_list, bias, out_tags, hilo):
            outs = [[npool.tile([128, YB], BF16, tag=tg, name=tg)
                     for tg in tgs] for tgs in out_tags]
            with tc.tile_pool(name=f"At{out_tags[0][0]}", bufs=2) as tp, \
                 tc.tile_pool(name=f"Ap{out_tags[0][0]}", bufs=2,
                              space="PSUM") as pp:
                for yg in range(8):
                    y0 = 2 * yg
                    ps = [pp.tile([128, 2 * BL], F32, tag=f"p{i}",
                                  name=f"p{i}") for i in range(2)]
                    mm = []
                    for dy in (0, -1, 1, -2, 2):
                        cl = _clip_dy(y0, 2, dy)
                        if cl is None:
                            continue
                        n0, N, ysrc = cl
                        for ops in op_list:
                            for g in range(4):
                                mm.append((ops[dy + 2][g], g, n0, N, ysrc,
                                           N // BL))
                    if yg == 0:
                        for ops in op_list:
                            for g in range(4):
                                mm.append((ops[5][g], g, 0, BL, 0, 1))
                    if yg == 7:
                        for ops in op_list:
                            for g in range(4):
                                mm.append((ops[6][g], g, BL, BL, 15, 1))
                    first_g, last_g = {}, {}
                    for i, (op, g, n0, N, ysrc, nys) in enumerate(mm):
                        first_g.setdefault(g, i)
                        last_g[g] = i
                    for i, (op, g, n0, N, ysrc, nys) in enumerate(mm):
                        nc.tensor.matmul(
                            ps[g // 2][64 * (g % 2):64 * (g % 2) + 64,
                                       n0:n0 + N],
                            op[0:120, :],
                            wins[g][0:120, ysrc:ysrc + nys, :],
                            start=(i == first_g[g]), stop=(i == last_g[g]),
                            tile_position=(0, 64 * (g % 2)))
                    sl = slice(y0 * BL, (y0 + 2) * BL)
                    for i in range(2):
                        if hilo:
                            tmp = tp.tile([128, 2 * BL], F32, tag=f"t{i}",
                                          name=f"t{i}")
                            nc.scalar.activation(tmp[:], ps[i][:],
                                                 AF.Relu, bias=bias[:])
                            nc.scalar.activation(outs[0][i][:, sl],
                                                 ps[i][:], AF.Relu,
                                                 bias=bias[:])
                            nc.vector.tensor_sub(outs[1][i][:, sl],
                                                 tmp[:], outs[0][i][:, sl])
                        else:
                            nc.scalar.activation(outs[0][i][:, sl],
                                                 ps[i][:], AF.Relu,
                                                 bias=bias[:])
            return outs

        # -------- c2 window build (full-y, per g): 2-chunk src -> 1 tile ---
        def build_win_c2(src2, g, wp, tag, eng):
            w = wp.tile([128, H, BL], BF16, tag=tag, name=tag)
            x0 = 4 * g - 1
            if g == 0:
                nc.vector.memset(w[0:32, :, :], 0.0)
            if g == 3:
                nc.vector.memset(w[64:96, :, :], 0.0)
            xs_s, xs_e = max(0, x0), min(W, x0 + 6)
            if xs_s < 8 < xs_e:
                pieces = [(xs_s, 8), (8, xs_e)]
            else:
                pieces = [(xs_s, xs_e)]
            for (a, bb) in pieces:
                ch = a // 8
                eng.dma_start(
                    w[(a - x0) * 16:(bb - x0) * 16, :, :],
                    src2[ch].rearrange("p (y i) -> p y i", y=H)
                    [(a % 8) * 16:(a % 8) * 16 + (bb - a) * 16, :, :])
            return w

        # ---------------- c2 conv (3x3, window-6, M=128, g-outer) ----------
        def c2_conv(srcs, op_list, bias, out_tags, hilo):
            outs = [[npool.tile([128, YB], BF16, tag=tg, name=tg)
                     for tg in tgs] for tgs in out_tags]
            with tc.tile_pool(name=f"Cw{out_tags[0][0]}", bufs=2) as wp, \
                 tc.tile_pool(name=f"Ct{out_tags[0][0]}", bufs=2) as tp, \
                 tc.tile_pool(name=f"Cp{out_tags[0][0]}", bufs=2,
                              space="PSUM") as pp:
                for g in range(4):
                    winH = build_win_c2(srcs[0], g, wp, "hw", nc.sync)
                    passes = [(op_list[0], winH)]
                    if hilo:
                        winL = build_win_c2(srcs[1], g, wp, "lw", nc.scalar)
                        passes += [(op_list[1], winH), (op_list[0], winL)]
                    for yg in range(8):
                        y0 = 2 * yg
                        ps = pp.tile([128, 2 * BL], F32, tag="p", name="p")
                        mm = []
                        for dy in (0, -1, 1):
                            cl = _clip_dy(y0, 2, dy)
                            if cl is None:
                                continue
                            n0, N, ysrc = cl
                            for (ops, win) in passes:
                                mm.append((ops[dy + 1][g], win, n0, N, ysrc,
                                           N // BL))
                        for i, (op, win, n0, N, ysrc, nys) in enumerate(mm):
                            nc.tensor.matmul(
                                ps[:, n0:n0 + N],
                                op[0:96, :], win[0:96, ysrc:ysrc + nys, :],
                                start=(i == 0), stop=(i == len(mm) - 1))
                        sl = slice(y0 * BL, (y0 + 2) * BL)
                        if hilo:
                            tmp = tp.tile([128, 2 * BL], F32, tag="t",
                                          name="t")
                            nc.scalar.activation(tmp[:], ps[:], AF.Relu,
                                                 bias=bias[:])
                            nc.scalar.activation(outs[0][g][:, sl], ps[:],
                                                 AF.Relu, bias=bias[:])
                            nc.vector.tensor_sub(outs[1][g][:, sl], tmp[:],
                                                 outs[0][g][:, sl])
                        else:
                            nc.scalar.activation(outs[0][g][:, sl], ps[:],
                                                 AF.Relu, bias=bias[:])
            return outs

        # ================== tower 2 ==================
        with nc.named_scope("t2A"):
            X2h, X2l = stageA(wins_t2, [opsA_t2h, opsA_t2l],
                              bias_sb["b_c1t2"],
                              [["A0", "A1"], ["B0", "B1"]], hilo=True)
        with nc.named_scope("t2c2"):
            X3h, X3l = c2_conv([X2h, X2l], [opsC_t2h, opsC_t2l],
                               bias_sb["b_c2t2"],
                               [["C0", "C1", "C2", "C3"],
                                ["D0", "D1", "D2", "D3"]], hilo=True)

        # prefetch t1 stage-A windows while the scores stream runs
        # (reuses the X2 hi/lo buffers, dead after t2c2)
        wins_t1 = load_winA(ohs_d, ["A0", "A1", "B0", "B1"], [nc.gpsimd])

        # -------- fused scores: X3 @ M2 + brow; argmax -> codes ------------
        with nc.named_scope("t2sc"):
            with tc.tile_pool(name="m2p", bufs=8) as mwp, \
                 tc.tile_pool(name="scp", bufs=1) as scp, \
                 tc.tile_pool(name="spp", bufs=1, space="PSUM") as spp:
                sps = [spp.tile([128, NZ], F32, tag=f"s{m}", name=f"s{m}")
                       for m in range(2)]
                for k in range(64):
                    g, y = k // 16, k % 16
                    mh = mwp.tile([128, NZ], BF16, tag="mh", name="mh")
                    ml = mwp.tile([128, NZ], BF16, tag="ml", name="ml")
                    nc.sync.dma_start(mh[:], M2h_d[k])
                    nc.scalar.dma_start(ml[:], M2l_d[k])
                    for m in range(2):
                        c0 = y * BL + 128 * m
                        nc.tensor.matmul(sps[m][:],
                                         X3h[g][:, c0:c0 + 128], mh[:],
                                         start=(k == 0), stop=False)
                        nc.tensor.matmul(sps[m][:],
                                         X3l[g][:, c0:c0 + 128], mh[:],
                                         start=False, stop=False)
                        nc.tensor.matmul(sps[m][:],
                                         X3h[g][:, c0:c0 + 128], ml[:],
                                         start=False, stop=False)
                for m in range(2):
                    nc.tensor.matmul(sps[m][:], ones_k[:],
                                     bl_sb["brow"][:, 0:NZ], start=False,
                                     stop=True)
                idxs = []
                for m in range(2):
                    sc = scp.tile([128, NZ], F32, tag=f"sc{m}", name=f"sc{m}")
                    nc.vector.tensor_copy(sc[:], sps[m][:])
                    mx = scp.tile([128, 8], F32, tag=f"mx{m}", name=f"mx{m}")
                    nc.vector.max(mx[:], sc[:])
                    ix = scp.tile([128, 8], U32, tag=f"ix{m}", name=f"ix{m}")
                    nc.vector.max_index(ix[:], mx[:], sc[:])
                    idxs.append(ix)
                for m in range(2):
                    nc.gpsimd.dma_start(codes_loc_d[128 * m:128 * m + 128],
                                        idxs[m][:, 0:1])
            nc.gpsimd.collective_compute(
                "AllGather", mybir.AluOpType.bypass,
                replica_groups=[list(range(NCORES))],
                ins=[codes_loc_d[:]], outs=[codes_g_d[:]])

        # ================== tower 1 (bf16) ==================
        with nc.named_scope("t1A"):
            (Y2,) = stageA(wins_t1, [opsA_t1], bias_sb["b_c1t1"],
                           [["D0", "D1"]], hilo=False)
        with nc.named_scope("t1c2"):
            (Y3,) = c2_conv([Y2, None], [opsC_t1], bias_sb["b_c2t1"],
                            [["C0", "C1", "C2", "C3"]], hilo=False)

        # ---- gather zn rows by global codes (DMA only; overlaps t1) -------
        zrp = ES.enter_context(tc.tile_pool(name="zrp", bufs=1))
        codes_sb = zrp.tile([128, 16], U32, tag="codes", name="codes")
        nc.gpsimd.dma_start(codes_sb[:],
                            codes_g_d.rearrange("(c p) -> p c", p=128))
        zrs = []
        for c in range(16):
            zr = zrp.tile([128, ESZ], BF16, tag=f"zr{c}", name=f"zr{c}")
            nc.gpsimd.indirect_dma_start(
                out=zr[:], out_offset=None, in_=znb_d[:],
                in_offset=bass.IndirectOffsetOnAxis(
                    ap=codes_sb[:, c:c + 1], axis=0))
            zrs.append(zr)

        # ---------------- t1 linear -> embT1 (img, ESZ) --------------------
        with nc.named_scope("t1lin"):
            embT1 = [epool.tile([128, ESZ], F32, tag=f"e1T{m}",
                                name=f"e1T{m}") for m in range(2)]
            with tc.tile_pool(name="lwp", bufs=10) as lwp, \
                 tc.tile_pool(name="lpp", bufs=1, space="PSUM") as lpp:
                ps = [lpp.tile([128, ESZ], F32, tag=f"p{m}", name=f"p{m}")
                      for m in range(2)]
                for k in range(64):
                    g, y = k // 16, k % 16
                    lwt = lwp.tile([128, ESZ], BF16, tag="lw", name="lw")
                    (nc.sync if k % 2 == 0 else nc.scalar).dma_start(
                        lwt[:], lw1_d[k])
                    for m in range(2):
                        c0 = y * BL + 128 * m
                        nc.tensor.matmul(ps[m][:], Y3[g][:, c0:c0 + 128],
                                         lwt[:], start=(k == 0), stop=False)
                for m in range(2):
                    nc.tensor.matmul(ps[m][:], ones_k[:], bl_sb["b_l1"][:],
                                     start=False, stop=True)
                for m in range(2):
                    nc.scalar.activation(embT1[m][:], ps[m][:], AF.Identity)

            # rnt = exp(scale) / (|e1| + eps); scale embT1 rows in place
            with tc.tile_pool(name="nrm", bufs=1) as nrp:
                for m in range(2):
                    sq = nrp.tile([128, ESZ], F32, tag="sq", name="sq")
                    nc.vector.tensor_mul(sq[:], embT1[m][:], embT1[m][:])
                    n2 = nrp.tile([128, 1], F32, tag="n2", name="n2")
                    nc.vector.tensor_reduce(n2[:], sq[:],
                                            mybir.AxisListType.X,
                                            mybir.AluOpType.add)
                    nc.scalar.sqrt(n2[:], n2[:])
                    nc.vector.tensor_scalar_add(n2[:], n2[:], EPS)
                    nc.vector.reciprocal(n2[:], n2[:])
                    nc.vector.tensor_scalar_mul(n2[:], n2[:], esc)
                    nc.vector.tensor_scalar_mul(embT1[m][:], embT1[m][:],
                                                n2[:])

            # transpose to e1b (E, img) bf16; then zr chunks -> zT (E, B)
            e1b = [epool.tile([128, BL], BF16, tag=f"e1b{e}", name=f"e1b{e}")
                   for e in range(4)]
            zT = [epool.tile([128, B], BF16, tag=f"zT{e}", name=f"zT{e}")
                  for e in range(4)]
            with tc.tile_pool(name="tpq", bufs=2) as tpq, \
                 tc.tile_pool(name="tpp", bufs=2, space="PSUM") as tpp:
                for m in range(2):
                    for e in range(4):
                        tp = tpp.tile([128, 128], F32, tag="tp", name="tp")
                        nc.tensor.transpose(tp[:],
                                            embT1[m][:, 128 * e:128 * e + 128],
                                            ident_sb[:])
                        nc.vector.tensor_copy(e1b[e][:, 128 * m:128 * m + 128],
                                              tp[:])
                for c in range(16):
                    zf = tpq.tile([128, ESZ], F32, tag="zf", name="zf")
                    nc.vector.tensor_copy(zf[:], zrs[c][:])
                    for e in range(4):
                        tp = tpp.tile([128, 128], F32, tag="tp", name="tp")
                        nc.tensor.transpose(tp[:],
                                            zf[:, 128 * e:128 * e + 128],
                                            ident_sb[:])
                        nc.vector.tensor_copy(
                            zT[e][:, 128 * c:128 * c + 128], tp[:])

        # ---------------- final: out = (e1n @ zT-gathered) -----------------
        with nc.named_scope("fin"):
            with tc.tile_pool(name="fob", bufs=2) as fob, \
                 tc.tile_pool(name="fpp", bufs=2, space="PSUM") as fpp:
                for m in range(2):
                    for n in range(4):
                        fp = fpp.tile([128, 512], F32, tag="fp", name="fp")
                        for e in range(4):
                            nc.tensor.matmul(
                                fp[:], e1b[e][:, 128 * m:128 * m + 128],
                                zT[e][:, 512 * n:512 * n + 512],
                                start=(e == 0), stop=(e == 3))
                        ob = fob.tile([128, 512], F32, tag="ob", name="ob")
                        nc.scalar.activation(ob[:], fp[:], AF.Identity)
                        (nc.sync if n % 2 == 0 else nc.scalar).dma_start(
                            out_d[128 * m:128 * m + 128,
                                  512 * n:512 * n + 512], ob[:])

    nc.compile()
    return nc


def make_in_maps(shared, percore):
    maps = []
    for pc in percore:
        m = dict(shared)
        m["ohs"] = pc["ohs"]
        m["ohd"] = pc["ohd"]
        maps.append(m)
    return maps


def _run(inputs, trace=False):
    dsf = np.asarray(inputs.get("downscale_factor", 1)).reshape(-1)
    dsf = int(dsf[0]) if dsf.size else 1
    assert dsf == 1, f"only downscale_factor=1 supported, got {dsf}"
    shared, percore, esc = host_prep(inputs)
    nc = build_program(esc)
    maps = make_in_maps(shared, percore)
    res = run_bass_kernel_spmd(nc, maps, list(range(NCORES)), trace=trace)
    out = np.concatenate(
        [res.results[c]["out"] for c in range(NCORES)], axis=0)
    return np.ascontiguousarray(out, np.float32), res


def kernel(**inputs):
    out, _ = _run(inputs, trace=False)
    return out


def run_for_test(inputs, trace=False):
    return _run(inputs, trace=trace)
